# revision 19
# baseline (speedup 1.0000x reference)
"""Coref mention-ranking head on 8 TRN2 NeuronCores (Bass/Tile).

Math (reference): for mention i and antecedent slot c in [0, 50):
    J = max(0, i-50) + c, valid iff c < min(i, 50)
    combined = [cur_i, ant_J, cur_i*ant_J, dist_emb[clip(i-J,0,9)]]
    score = relu(combined @ W1 + b1) @ W2 + b2
    out[i, 0] = 0; out[i, c+1] = score (masked to 0 if invalid)

Decomposition used here (s = i - J in [1, 50] is the "shift"):
    z(i,s) = (cur_i*ant_{i-s}) @ W1c + ZA[i] + ZB[i-s] + zdf[min(s,9)] + b1
    score(i,s) = sign(W2) . relu(|W2| * z(i,s))        (|W2| folded into W1/b1)
The per-mention linear terms ZA = emb @ W1a, ZB = emb @ W1b and the
per-shift zdf = dist_emb @ W1d + b1 are cheap (O(N*H*H) once) and are
precomputed on the HOST; the device runs only the per-pair band GEMM
(N*A*H*H, 99% of the FLOPs) plus the additive fixup and the scorer.
Device computes the dense grid score[s, i]; host scatters it into slots.

Sharding: mention axis split across 8 cores (256 mentions each); W1c and
the small tables replicated. Each core receives a 306-column transposed
embedding window [n0-50, n0+256) (zero-padded for core 0).

Perf notes: the band GEMM and the X=cur*ant products run in bf16 (fp32r
is 2 PE passes; fp8 fails the 2e-2 gate - measured 2.6e-2 rel err). A
column-shifted copy of embT keeps the odd shifts 4B-aligned so the bf16
X-products hit the DVE 2x mode. Additive terms go in AFTER the matmuls
(PE->DVE->ACT ordering; DVE-prefill-then-matmul-accumulate races in the
tile scheduler): one scalar_tensor_tensor adds both shift segments' ZB
via a stride-paired host table, one tensor_add adds ZA broadcast, and the
per-shift zdf rides the ACT relu's per-partition bias port. The score
matmuls for a whole block run back-to-back one m-group after the block's
last relu: switching the PE stationary between 128-wide w1c tiles and the
1-wide sgn vector costs ~95ns per transition, so batching leaves 2
transitions per block instead of 16. DMA emission order = arrival order:
small tables, embT/embT1 (unblocks the DVE X-products), w1c (unblocks the
main matmuls), then the per-m ZA/ZB chunks which land while block 0 runs.
"""

from contextlib import ExitStack

import numpy as np

import concourse.bass as bass
import concourse.bacc as bacc
import concourse.tile as tile
from concourse import mybir
from concourse.bass_utils import run_bass_kernel_spmd

F32 = mybir.dt.float32
BF16 = mybir.dt.bfloat16
RELU = mybir.ActivationFunctionType.Relu

N = 2048      # mentions
H = 1024      # hidden
A = 50        # max antecedents
FEAT = 20
NCORES = 8
NLOC = N // NCORES          # 256 mentions per core
W = NLOC + A                # 306-column embedding window per core
KT = H // 128               # 8 h_in tiles
MT = H // 128               # 8 h_out tiles
NBLK = A // 2               # 25 blocks of 2 shifts x 256 mentions = 512 pairs


def _build_nc():
    nc = bacc.Bacc("TRN2", target_bir_lowering=False, debug=False)

    embT_d = nc.dram_tensor("embT", [H, W], BF16, kind="ExternalInput")
    w1c_d = nc.dram_tensor("w1c", [H, H], BF16, kind="ExternalInput")
    # sgn padded to 128-wide stationary tiles (col 0 = sign(W2), rest 0) so
    # the score matmuls keep the same PE tile config as the w1c matmuls --
    # a (128,128)->(128,1) stationary switch costs ~95ns in reconfig.
    sgn_d = nc.dram_tensor("sgn", [128, MT * 128], BF16, kind="ExternalInput")
    zdf_d = nc.dram_tensor("zdf", [128, MT * A], F32, kind="ExternalInput")
    zat_d = nc.dram_tensor("zat", [128, MT * W], F32, kind="ExternalInput")
    zbp_d = nc.dram_tensor("zbp", [128, MT * 2 * W], F32, kind="ExternalInput")
    scores_d = nc.dram_tensor("scores", [NBLK, 512], F32, kind="ExternalOutput")

    with tile.TileContext(nc) as tc, ExitStack() as ctx:
        const = ctx.enter_context(tc.tile_pool(name="const", bufs=1))
        xpool = ctx.enter_context(tc.tile_pool(name="x", bufs=3))
        htpool = ctx.enter_context(tc.tile_pool(name="ht", bufs=8))

        # DMA emission order = arrival order (single queue): small tables,
        # embT/embT1 (X-products), w1c (main matmuls), then ZA/ZB per-m
        # chunks, which land while the first block's matmuls run.
        sgn_sb = const.tile([128, MT, 128], BF16)
        nc.sync.dma_start(sgn_sb[:], sgn_d[:])
        zdfb1 = const.tile([128, MT, A], F32)
        nc.sync.dma_start(zdfb1[:], zdf_d[:])

        embT = const.tile([128, KT, W], BF16)
        for k in range(KT):
            nc.sync.dma_start(embT[:, k, :], embT_d[k * 128:(k + 1) * 128, :])
        # w1c in two m-halves: the first 4 m-groups' weights land ~4us
        # sooner, letting the main matmuls start while the rest streams.
        w1c_sb = const.tile([128, KT, H], BF16)
        for half in range(2):
            lo, hi = half * (H // 2), (half + 1) * (H // 2)
            for k in range(KT):
                nc.sync.dma_start(w1c_sb[:, k, lo:hi],
                                  w1c_d[k * 128:(k + 1) * 128, lo:hi])
        # column-shifted copy: embT1[:, k, w] = embT[:, k, w+1]; keeps the
        # odd-shift X-product reads 4B-aligned (DVE 2x mode needs it).
        # Built on-device (cheap DVE copies) instead of spending pre-main
        # DMA bandwidth on a second 0.6MB transfer.
        embT1 = const.tile([128, KT, W], BF16)
        for k in range(KT):
            nc.vector.tensor_copy(embT1[:, k, 0:W - 1], embT[:, k, 1:W])

        ZAT = const.tile([128, MT, W], F32)
        # ZBP[:, m, 0, w] = ZB[w], ZBP[:, m, 1, w] = ZB[w-1]: the pair view
        # lets one STT add ZB for both shift segments of a block.
        ZBP = const.tile([128, MT, 2, W], F32)
        for m in range(MT):
            nc.sync.dma_start(ZAT[:, m, :], zat_d[:, m * W:(m + 1) * W])
            nc.sync.dma_start(ZBP[:, m, :, :],
                              zbp_d[:, m * 2 * W:(m + 1) * 2 * W])

        # Main loop: block b covers shifts s0=2b+1, s0+1, each over the 256
        # local mentions -> 512 pairs in the moving dimension.
        psum_main = ctx.enter_context(
            tc.tile_pool(name="psum_main", bufs=7, space="PSUM"))
        ADD = mybir.AluOpType.add
        pending = None   # (blk, sps_ap, [ht aps]) score matmuls, 1 group late

        def flush_pending():
            blk, sps_ap, hts = pending
            for m in range(MT):
                nc.tensor.matmul(
                    sps_ap, sgn_sb[:, m, :], hts[m],
                    start=(m == 0), stop=(m == MT - 1),
                )
            srow = htpool.tile([1, 512], F32, name=f"srow{blk}",
                               tag="srow", bufs=2)
            nc.scalar.copy(srow[:], sps_ap[0:1, :])
            nc.sync.dma_start(scores_d[blk:blk + 1, :], srow[:])

        for b in range(NBLK):
            s0 = 2 * b + 1
            X = xpool.tile([128, KT, 512], BF16, name=f"X{b}", tag="X")
            for k in range(KT):
                # j=0: odd shift s0 -> shifted copy keeps the read aligned
                nc.vector.tensor_mul(
                    X[:, k, 0:256],
                    embT[:, k, A:W],
                    embT1[:, k, A - s0 - 1:A - s0 - 1 + 256],
                )
                # j=1: even shift s0+1 reads embT at even offset A-s0-1
                nc.vector.tensor_mul(
                    X[:, k, 256:512],
                    embT[:, k, A:W],
                    embT[:, k, A - s0 - 1:W - s0 - 1],
                )
            sps = psum_main.tile([128, 512], F32, name=f"sps{b}", tag="sps",
                                 bufs=2)
            hts = []
            for m in range(MT):
                ps = psum_main.tile([128, 512], F32, name=f"ps{b}_{m}",
                                    tag="ps", bufs=5)
                for k in range(KT):
                    nc.tensor.matmul(
                        ps[:],
                        w1c_sb[:, k, m * 128:(m + 1) * 128],
                        X[:, k, :],
                        start=(k == 0), stop=(k == KT - 1),
                    )
                if m == 1 and pending is not None:
                    flush_pending()
                ps2 = ps[:].rearrange("p (j i) -> p j i", j=2)
                # both shift segments' ZB in one pass via the pair view
                # (segment j=1 is ZB shifted one column), then ZA broadcast.
                nc.vector.scalar_tensor_tensor(
                    ps2, ps2, 0.0,
                    ZBP[:, m, :, A - s0:A - s0 + 256],
                    ADD, ADD,
                )
                nc.vector.tensor_add(
                    ps2, ps2,
                    ZAT[:, m:m + 1, A:W].broadcast_to([128, 2, 256]),
                )
                ht = htpool.tile([128, 512], BF16, name=f"ht{b}_{m}",
                                 tag="ht", bufs=18)
                # relu(z + zdf_s): per-shift zdf rides the ACT bias port
                for j in range(2):
                    s = s0 + j
                    nc.scalar.activation(
                        ht[:, j * 256:(j + 1) * 256],
                        ps[:, j * 256:(j + 1) * 256],
                        RELU, bias=zdfb1[:, m, s - 1:s],
                    )
                hts.append(ht[:])
            pending = (b, sps[:], hts)
        flush_pending()

    nc.compile()
    if not nc.is_finalized():
        nc.finalize()
    return nc


def _host_prep(mention_embeddings, W1, b1, W2, dist_emb):
    import ml_dtypes
    bf16 = ml_dtypes.bfloat16

    emb = np.asarray(mention_embeddings, dtype=np.float32)
    W1 = np.asarray(W1, dtype=np.float32)
    b1 = np.asarray(b1, dtype=np.float32)
    W2 = np.asarray(W2, dtype=np.float32)
    dist_emb = np.asarray(dist_emb, dtype=np.float32)

    absw = np.abs(W2)
    sgn = np.sign(W2).astype(np.float32)
    W1s = W1 * absw[None, :]
    b1s = b1 * absw

    w1c = np.ascontiguousarray(W1s[2 * H:3 * H]).astype(bf16)
    # sgn padded: for tile m, a [128,128] stationary whose col 0 holds the
    # signs for hidden units m*128..m*128+127 and whose other cols are 0.
    sgnP = np.zeros((128, MT, 128), np.float32)
    sgnP[:, :, 0] = sgn.reshape(MT, 128).T
    sgn_in = np.ascontiguousarray(sgnP.reshape(128, MT * 128)).astype(bf16)

    # padded transposed embeddings: global col g holds mention g - A
    embTfull = np.zeros((H, N + A), dtype=np.float32)
    embTfull[:, A:] = emb.T
    embq = embTfull.astype(bf16).astype(np.float32)    # device bf16 rounding

    # host-side linear terms (bf16 inputs, fp32 accumulate = device numerics)
    w1a = W1s[0:H].astype(bf16).astype(np.float32)
    w1b = W1s[H:2 * H].astype(bf16).astype(np.float32)
    ZAg = embq.T @ w1a                                  # [N+A, H]
    ZBg = embq.T @ w1b
    ZBg0 = np.vstack([np.zeros((1, H), np.float32), ZBg])  # ZBg0[g] = ZB[g-1]

    # zdf[s-1, h] = dist_emb[min(s,9)] @ W1d*|W2| + b1*|W2|
    svals = np.arange(1, A + 1)
    zdf = dist_emb[np.minimum(svals, 9)] @ W1s[3 * H:] + b1s   # [A, H]
    # device layout [128, MT, A]: zdf_dev[p, m, s-1] = zdf[s-1, m*128+p]
    zdf_dev = np.ascontiguousarray(
        zdf.T.reshape(MT, 128, A).transpose(1, 0, 2).reshape(128, MT * A))

    in_maps = []
    for r in range(NCORES):
        n0 = r * NLOC
        # ZAT[p, m, w] = ZA[window col w, h=m*128+p]
        zat = ZAg[n0:n0 + W].T.reshape(MT, 128, W).transpose(1, 0, 2)
        # ZBP[p, m, 0, w] = ZB[col w]; ZBP[p, m, 1, w] = ZB[col w-1]
        zb0 = ZBg[n0:n0 + W]                   # [W, H]
        zb1 = ZBg0[n0:n0 + W]
        zbp = np.stack([zb0, zb1], axis=0)     # [2, W, H]
        zbp = zbp.transpose(2, 0, 1).reshape(MT, 128, 2 * W).transpose(1, 0, 2)
        in_maps.append({
            "embT": np.ascontiguousarray(embTfull[:, n0:n0 + W]).astype(bf16),
            "w1c": w1c,
            "sgn": sgn_in,
            "zdf": zdf_dev,
            "zat": np.ascontiguousarray(zat.reshape(128, MT * W)),
            "zbp": np.ascontiguousarray(zbp.reshape(128, MT * 2 * W)),
        })
    return in_maps


def _assemble(grids, b2):
    """grids: list of 8 per-core [NBLK, 512] score arrays -> [N, A+1] output."""
    b2v = np.float32(np.asarray(b2).reshape(-1)[0])
    # [50, 2048]: grid[s-1, i]
    grid = np.concatenate([np.asarray(g, dtype=np.float32).reshape(A, NLOC)
                           for g in grids], axis=1)
    out = np.zeros((N, A + 1), dtype=np.float32)
    big = grid[::-1].T + b2v          # big[i, c] = score(i, s=50-c) + b2
    out[A:, 1:] = big[A:]
    for i in range(1, A):
        ss = np.arange(1, i + 1)      # valid shifts for mention i < 50
        out[i, 1 + (i - ss)] = grid[ss - 1, i] + b2v
    return out


def kernel(mention_embeddings, mention_indices, max_antecedents, W1, b1, W2,
           b2, dist_emb):
    assert int(max_antecedents) == A
    in_maps = _host_prep(mention_embeddings, W1, b1, W2, dist_emb)
    nc = _build_nc()
    res = run_bass_kernel_spmd(nc, in_maps, list(range(NCORES)))
    grids = [res.results[r]["scores"] for r in range(NCORES)]
    return _assemble(grids, b2)
